# revision 1
# baseline (speedup 1.0000x reference)
"""CrossWinAttention Trainium2 kernel.

Full inputs in, full output out. Shards the 128 independent attention windows
(b=2 x x=8 x y=8) across 8 NeuronCores (16 windows each), runs a Bass/Tile
kernel per core, gathers the result.

Per-window math (Nq = Nk = 256 tokens, d = 128, 4 heads x 32):
  LN(q|k|v) -> QKV projections -> per-head softmax(QK^T/sqrt(32))V -> mean
  over n -> output projection.

Device dataflow per window (16 windows/core, fully unrolled, Tile-scheduled):
  DMA 6 token-major chunks -> bn_stats/bn_aggr -> rstd via ACT ln/exp ->
  LN apply (DVE, out bf16) -> PE transpose -> QKV matmuls (bf16) ->
  scores built transposed S^T[k,q] (row-tiled K=32 matmuls) -> ACT exp ->
  PE column-sums (col-tiled) -> reciprocal -> broadcast matmul B ->
  PV (col-tiled, kc-accumulated) -> aT*B -> n-mean -> output matmul -> DMA.
"""

import math
import numpy as np
import ml_dtypes

import concourse.bass as bass
import concourse.tile as tile
from concourse import mybir
from concourse.bass_utils import run_bass_kernel_spmd

F32 = mybir.dt.float32
BF16 = mybir.dt.bfloat16
AF = mybir.ActivationFunctionType
ALU = mybir.AluOpType

HEADS, DH, D = 4, 32, 128
INNER = HEADS * DH
EPS = 1e-5
SCALE = DH ** -0.5
NW = 16          # windows per core
NPAIR = 2        # (b,x) pairs per core
NY = 8

_BUILD_CACHE = {}


def _split_multi_waits(module):
    """This container's walrus rejects instructions with >1 sync wait. Engines
    execute in order, so hoist extra waits onto preceding NoOps."""
    import copy
    import bass_rust

    for function in module.functions:
        new_blocks = []
        for block in function.blocks:
            insts = []
            for inst in block.instructions:
                si = getattr(inst, "sync_info", None)
                waits = list(si.on_wait) if si is not None and si.on_wait else []
                if len(waits) > 1:
                    for k, w in enumerate(waits[:-1]):
                        insts.append(mybir.InstNoOp(
                            name=f"{inst.name}-w{k}",
                            engine=inst.engine,
                            ins=[], outs=[],
                            sync_info=bass_rust.SyncInfo(
                                on_wait=[w], on_update=[]),
                        ))
                    inst = copy.replace(
                        inst,
                        sync_info=bass_rust.SyncInfo(
                            on_wait=[waits[-1]], on_update=list(si.on_update)),
                    )
                insts.append(inst)
            new_blocks.append(copy.replace(block, instructions=insts))
        function.blocks.clear()
        for nb in new_blocks:
            function.blocks.append(nb)
    return module


def _build_program(add_cq, add_ck, add_cv, add_bp, split=True, stage=4, reps=1, loop=1):
    key = (add_cq, add_ck, add_cv, add_bp, split, stage, reps, loop)
    if key in _BUILD_CACHE:
        return _BUILD_CACHE[key]

    nc = bass.Bass()

    # DRAM I/O (per core)
    xq_d = nc.dram_tensor("xq", [NPAIR, 4, NY, 8, 8, D], F32, kind="ExternalInput")
    xk_d = nc.dram_tensor("xk", [NPAIR, 4, NY, 8, 8, D], F32, kind="ExternalInput")
    xv_d = nc.dram_tensor("xv", [NPAIR, 4, NY, 8, 8, D], F32, kind="ExternalInput")
    wqp_d = nc.dram_tensor("wqp", [D, 4, INNER], BF16, kind="ExternalInput")
    wk_d = nc.dram_tensor("wk", [D, INNER], BF16, kind="ExternalInput")
    wv_d = nc.dram_tensor("wv", [D, INNER], BF16, kind="ExternalInput")
    wp_d = nc.dram_tensor("wp", [INNER, D], BF16, kind="ExternalInput")
    ident_d = nc.dram_tensor("ident", [128, 128], BF16, kind="ExternalInput")
    ones_d = nc.dram_tensor("onesc", [128, 32], BF16, kind="ExternalInput")
    cq_d = nc.dram_tensor("cq", [INNER, 1], F32, kind="ExternalInput") if add_cq else None
    ck_d = nc.dram_tensor("ck", [INNER, 1], F32, kind="ExternalInput") if add_ck else None
    cv_d = nc.dram_tensor("cv", [INNER, 1], F32, kind="ExternalInput") if add_cv else None
    bp_d = nc.dram_tensor("bpb", [64, D], F32, kind="ExternalInput") if add_bp else None
    z_d = nc.dram_tensor("z", [NPAIR, NY, 8, 8, D], F32, kind="ExternalOutput")

    with tile.TileContext(nc) as tc:
        with (
            tc.tile_pool(name="const", bufs=1) as constp,
            tc.tile_pool(name="xin", bufs=3) as xin,
            tc.tile_pool(name="xln", bufs=2) as xlnp,
            tc.tile_pool(name="stat", bufs=3) as statp,
            tc.tile_pool(name="big", bufs=2) as bigp,
            tc.tile_pool(name="exps", bufs=2) as expp,
            tc.tile_pool(name="sml", bufs=3) as smlp,
            tc.tile_pool(name="ps_big", bufs=1, space="PSUM") as ps_big,
            tc.tile_pool(name="ps_chain", bufs=2, space="PSUM") as ps_chain,
            tc.tile_pool(name="ps_pv", bufs=1, space="PSUM") as ps_pv,
        ):
            # constants
            wqp_sb = constp.tile([D, 4, INNER], BF16, tag="wqp")
            wk_sb = constp.tile([D, INNER], BF16, tag="wk")
            wv_sb = constp.tile([D, INNER], BF16, tag="wv")
            wp_sb = constp.tile([INNER, D], BF16, tag="wp")
            ident = constp.tile([128, 128], BF16, tag="ident")
            onesc = constp.tile([128, 32], BF16, tag="onesc")
            eps_t = constp.tile([128, 1], F32, tag="eps")
            nc.vector.memset(eps_t, EPS)
            nc.sync.dma_start(out=wqp_sb, in_=wqp_d[:, :, :])
            nc.sync.dma_start(out=wk_sb, in_=wk_d[:, :])
            nc.sync.dma_start(out=wv_sb, in_=wv_d[:, :])
            nc.sync.dma_start(out=wp_sb, in_=wp_d[:, :])
            nc.sync.dma_start(out=ident, in_=ident_d[:, :])
            nc.sync.dma_start(out=onesc, in_=ones_d[:, :])
            cq_sb = ck_sb = cv_sb = bp_sb = None
            if add_cq:
                cq_sb = constp.tile([INNER, 1], F32, tag="cq")
                nc.sync.dma_start(out=cq_sb, in_=cq_d[:, :])
            if add_ck:
                ck_sb = constp.tile([INNER, 1], F32, tag="ck")
                nc.sync.dma_start(out=ck_sb, in_=ck_d[:, :])
            if add_cv:
                cv_sb = constp.tile([INNER, 1], F32, tag="cv")
                nc.sync.dma_start(out=cv_sb, in_=cv_d[:, :])
            if add_bp:
                bp_sb = constp.tile([64, D], F32, tag="bpb")
                nc.sync.dma_start(out=bp_sb, in_=bp_d[:, :])

            import contextlib
            loop_cm = tc.For_i(0, loop, 1) if loop > 1 else contextlib.nullcontext()
            with loop_cm:
             for rep in range(reps):
              for g in range(8):
                p, y0 = g // 4, 2 * (g % 4)
                # ---- one DMA per tensor per 2-window group ----
                x_all = xin.tile([128, 12, 128], F32, tag="xall")
                for t, src_d in enumerate((xq_d, xk_d, xv_d)):
                    for wi in range(2):
                        for c in range(2):
                            sap = bass.AP(
                                tensor=src_d,
                                offset=(p * 262144 + (y0 + wi) * 8192
                                        + 2 * c * 65536),
                                ap=[[65536, 2], [1, 8192]],
                            )
                            nc.sync.dma_start(
                                out=x_all[:, 6 * wi + 2 * t + c, :], in_=sap)

                zt = smlp.tile([128, D], F32, tag="zt")
                for wi in range(2):
                    xw = x_all[:, 6 * wi:6 * wi + 6, :]
                    # ---- LN stats: one multi-group bn_stats + manual combine
                    st6 = statp.tile([128, 6, 6], F32, tag="st6")
                    for i in range(6):
                        nc.vector.bn_stats(out=st6[:, i, :], in_=xw[:, i, :])
                    me, mo = st6[:, :, 1], st6[:, :, 4]
                    m2e, m2o = st6[:, :, 2], st6[:, :, 5]
                    ssum = statp.tile([128, 6], F32, tag="ssum")
                    nc.vector.tensor_add(out=ssum, in0=me, in1=mo)
                    mu = statp.tile([128, 6], F32, tag="mu")
                    nc.vector.tensor_scalar_mul(out=mu, in0=ssum, scalar1=0.5)
                    dmean = statp.tile([128, 6], F32, tag="dmean")
                    nc.vector.tensor_sub(out=dmean, in0=me, in1=mo)
                    d2 = statp.tile([128, 6], F32, tag="d2")
                    nc.vector.tensor_mul(out=d2, in0=dmean, in1=dmean)
                    sm2 = statp.tile([128, 6], F32, tag="sm2")
                    nc.vector.tensor_add(out=sm2, in0=m2e, in1=m2o)
                    v128 = statp.tile([128, 6], F32, tag="v128")
                    nc.vector.scalar_tensor_tensor(
                        out=v128, in0=d2, scalar=32.0, in1=sm2,
                        op0=ALU.mult, op1=ALU.add)
                    lnv = statp.tile([128, 6], F32, tag="lnv")
                    nc.scalar.activation(out=lnv, in_=v128, func=AF.Ln,
                                         bias=eps_t[:, 0:1], scale=1.0 / 128.0)
                    rstd = statp.tile([128, 6], F32, tag="rstd")
                    nc.scalar.activation(out=rstd, in_=lnv, func=AF.Exp,
                                         scale=-0.5)

                    # ---- LN apply -> bf16, PE transpose ----
                    psXT = ps_big.tile([128, 768], BF16, tag="psxtkv")
                    for i in range(6):
                        xl = xlnp.tile([128, D], BF16, tag=f"xl{i}")
                        nc.vector.tensor_scalar(
                            out=xl, in0=xw[:, i, :],
                            scalar1=mu[:, i:i + 1], scalar2=rstd[:, i:i + 1],
                            op0=ALU.subtract, op1=ALU.mult)
                        nc.tensor.transpose(
                            out=psXT[:, 128 * i:128 * (i + 1)], in_=xl,
                            identity=ident)
                    xT = bigp.tile([128, 768], BF16, tag="xT")
                    nc.scalar.copy(out=xT, in_=psXT)

                    # ---- projections ----
                    psQp = ps_big.tile([128, 1024], F32, tag="psqp")
                    for h in range(HEADS):
                        nc.tensor.matmul(out=psQp[:, 256 * h:256 * (h + 1)],
                                         lhsT=wqp_sb[:, h, :], rhs=xT[:, 0:256],
                                         start=True, stop=True)
                    psKV = ps_big.tile([128, 512], F32, tag="psxtkv")
                    nc.tensor.matmul(out=psKV[:, 0:256], lhsT=wk_sb,
                                     rhs=xT[:, 256:512], start=True, stop=True)
                    for c in range(2):
                        nc.tensor.matmul(
                            out=psKV[:, 256 + 128 * c:256 + 128 * (c + 1)],
                            lhsT=xT[:, 512 + 128 * c:512 + 128 * (c + 1)],
                            rhs=wv_sb, start=True, stop=True)
                    qpad = bigp.tile([128, 1024], BF16, tag="qpad")
                    nc.scalar.copy(out=qpad, in_=psQp)
                    qk = bigp.tile([128, 256], BF16, tag="qk")
                    if add_ck:
                        nc.scalar.activation(out=qk, in_=psKV[:, 0:256],
                                             func=AF.Copy, bias=ck_sb[:, 0:1])
                    else:
                        nc.scalar.copy(out=qk, in_=psKV[:, 0:256])
                    vtm = bigp.tile([128, 256], BF16, tag="vtm")
                    nc.vector.tensor_copy(out=vtm, in_=psKV[:, 256:512])

                    # ---- scores S^T + exp ----
                    expTs = []
                    for kc in range(2):
                        psS = ps_big.tile([128, 1024], F32, tag="pss")
                        for half in range(2):
                            nc.tensor.matmul(
                                out=psS[:, 512 * half:512 * (half + 1)],
                                lhsT=qk[:, 128 * kc:128 * (kc + 1)],
                                rhs=qpad[:, 512 * half:512 * (half + 1)],
                                start=True, stop=True)
                        expT = expp.tile([128, 1024], BF16, tag=f"expT{kc}")
                        nc.scalar.activation(out=expT, in_=psS, func=AF.Exp,
                                             scale=SCALE)
                        expTs.append(expT)

                    # ---- column sums (col-tiled) + reciprocal + B ----
                    psSum = ps_chain.tile([128, 256], F32, tag="chain")
                    for h in range(HEADS):
                        for kc in range(2):
                            nc.tensor.matmul(
                                out=psSum[32 * h:32 * (h + 1), :],
                                lhsT=onesc,
                                rhs=expTs[kc][:, 256 * h:256 * (h + 1)],
                                start=(kc == 0), stop=(kc == 1),
                                tile_position=(0, 32 * h),
                                skip_group_check=True)
                    vT = smlp.tile([128, 256], F32, tag="vT")
                    nc.vector.transpose(out=vT, in_=psSum)
                    rinv = smlp.tile([128, 8], F32, tag="rinv")
                    nc.vector.reciprocal(out=rinv, in_=vT[:, 0:256:32])
                    rinv_b = bass.AP(tensor=rinv.tensor, offset=rinv.offset,
                                     ap=list(rinv.ap) + [[0, 32]])
                    rexp = smlp.tile([128, 256], F32, tag="rexp")
                    nc.vector.tensor_copy(
                        out=rexp.rearrange("p (a b) -> p a b", a=8), in_=rinv_b)
                    Bsb = smlp.tile([128, 256], F32, tag="Bsb")
                    nc.vector.transpose(out=Bsb, in_=rexp)

                    # ---- PV (col-tiled, kc-accumulated) ----
                    psA = ps_pv.tile([128, 256], F32, tag="psa")
                    for h in range(HEADS):
                        for kc in range(2):
                            nc.tensor.matmul(
                                out=psA[32 * h:32 * (h + 1), :],
                                lhsT=vtm[:, 128 * kc + 32 * h:
                                         128 * kc + 32 * (h + 1)],
                                rhs=expTs[kc][:, 256 * h:256 * (h + 1)],
                                start=(kc == 0), stop=(kc == 1),
                                tile_position=(0, 32 * h),
                                skip_group_check=True)
                    aT = smlp.tile([128, 256], BF16, tag="aT")
                    nc.vector.tensor_tensor(out=aT, in0=psA, in1=Bsb,
                                            op=ALU.mult)
                    if add_cv:
                        nc.vector.tensor_scalar(out=aT, in0=aT,
                                                scalar1=cv_sb[:, 0:1],
                                                scalar2=None, op0=ALU.add)

                    # ---- mean over n ----
                    t1 = smlp.tile([128, 64], BF16, tag="t1")
                    t2 = smlp.tile([128, 64], BF16, tag="t2")
                    abar = smlp.tile([128, 64], BF16, tag="abar")
                    nc.gpsimd.tensor_tensor(out=t1, in0=aT[:, 0:64],
                                            in1=aT[:, 64:128], op=ALU.add)
                    nc.gpsimd.tensor_tensor(out=t2, in0=aT[:, 128:192],
                                            in1=aT[:, 192:256], op=ALU.add)
                    nc.gpsimd.tensor_tensor(out=abar, in0=t1, in1=t2,
                                            op=ALU.add)

                    # ---- output projection into half of zt ----
                    psZ = ps_pv.tile([128, D], F32, tag="psa")
                    nc.tensor.matmul(out=psZ[64 * wi:64 * (wi + 1), :],
                                     lhsT=abar, rhs=wp_sb, start=True,
                                     stop=True,
                                     tile_position=((0, 64) if wi else None))
                    if add_bp:
                        nc.vector.tensor_tensor(
                            out=zt[64 * wi:64 * (wi + 1), :],
                            in0=psZ[64 * wi:64 * (wi + 1), :], in1=bp_sb,
                            op=ALU.add)
                    else:
                        nc.vector.tensor_copy(
                            out=zt[64 * wi:64 * (wi + 1), :],
                            in_=psZ[64 * wi:64 * (wi + 1), :])
                nc.sync.dma_start(
                    out=z_d[p, y0:y0 + 2].rearrange(
                        "y w1 w2 d -> (y w1 w2) d"), in_=zt)

    if split:
        _split_multi_waits(nc.m)
    _BUILD_CACHE[key] = nc
    return nc


def kernel(q, k, v, ln_q_g, ln_q_b, ln_k_g, ln_k_b, ln_v_g, ln_v_b,
           Wq, bq, Wk, bk, Wv, bv, Wp, bp):
    q = np.asarray(q, np.float32)
    k = np.asarray(k, np.float32)
    v = np.asarray(v, np.float32)
    b, n, x, y, w1, w2, d = q.shape

    # fold LN gains into the weights; biases become per-channel consts
    Wq_f = np.asarray(ln_q_g)[:, None] * np.asarray(Wq)
    Wq_p = np.zeros((128, 4, 128), np.float32)
    for h in range(4):
        Wq_p[:, h, 32 * h:32 * (h + 1)] = Wq_f[:, 32 * h:32 * (h + 1)]
    Wq_p = Wq_p.astype(ml_dtypes.bfloat16)
    Wk_e = (np.asarray(ln_k_g)[:, None] * np.asarray(Wk)).astype(ml_dtypes.bfloat16)
    Wv_e = (np.asarray(ln_v_g)[:, None] * np.asarray(Wv)).astype(ml_dtypes.bfloat16)
    Wp_e = (np.asarray(Wp) * 0.25).astype(ml_dtypes.bfloat16)
    cq = (np.asarray(ln_q_b) @ np.asarray(Wq) + np.asarray(bq)).astype(np.float32)
    ck = (np.asarray(ln_k_b) @ np.asarray(Wk) + np.asarray(bk)).astype(np.float32)
    cv = (np.asarray(ln_v_b) @ np.asarray(Wv) + np.asarray(bv)).astype(np.float32)
    bpv = np.asarray(bp, np.float32)

    add_cq, add_ck = bool(np.any(cq)), bool(np.any(ck))
    add_cv, add_bp = bool(np.any(cv)), bool(np.any(bpv))

    ident = np.eye(128, dtype=ml_dtypes.bfloat16)
    onesc = np.ones((128, 32), ml_dtypes.bfloat16)

    nc = _build_program(add_cq, add_ck, add_cv, add_bp)

    in_maps = []
    for core in range(8):
        pairs = [2 * core, 2 * core + 1]  # flat (b, x) index
        def shard(t):
            s = np.stack([t[pi // x, :, pi % x] for pi in pairs])  # [2,n,y,w1,w2,d]
            return np.ascontiguousarray(s)
        m = {
            "xq": shard(q), "xk": shard(k), "xv": shard(v),
            "wqp": Wq_p, "wk": Wk_e, "wv": Wv_e, "wp": Wp_e,
            "ident": ident, "onesc": onesc,
        }
        if add_cq:
            m["cq"] = cq[:, None]
        if add_ck:
            m["ck"] = ck[:, None]
        if add_cv:
            m["cv"] = cv[:, None]
        if add_bp:
            m["bpb"] = np.tile(bpv[None, :], (64, 1))
        in_maps.append(m)

    import os
    r = run_bass_kernel_spmd(nc, in_maps, list(range(8)),
                             trace=bool(os.environ.get("KERNEL_TRACE")))
    global LAST
    LAST = {"exec_time_ns": r.exec_time_ns, "profile_json": r.profile_json,
            "instructions_and_trace": r.instructions_and_trace}
    res = r.results

    out = np.zeros((b, x, y, w1, w2, d), np.float32)
    for core in range(8):
        zc = res[core]["z"]  # [2, y, 8, 8, d]
        for j, pi in enumerate([2 * core, 2 * core + 1]):
            out[pi // x, pi % x] = zc[j]
    return out

